# revision 1
# baseline (speedup 1.0000x reference)
"""Trainium2 Bass kernel for nn_CascadedAttention (B=64, T=512, D=1024, V=28).

Math notes (why this is NOT a 512-step sequential scan on device):

  reference computes, per step t with carry y_prev (y_{-1} = 0):
    scores = softmax(tanh(...) @ Va, axis=-1)     # softmax over a SIZE-1 axis
                                                  # -> exactly 1.0 everywhere
    c      = einsum('btd,bt->bd', x, scores)      # -> x.sum(axis=1), step-invariant
    idx    = int32(y_prev)                        # y_prev in (0,1] -> idx in {0,1};
                                                  # idx==1 iff y_prev == 1.0 (fp32-saturated sigmoid)
    WoE    = emb_table[idx] @ Wo                  # -> w0 + (w1-w0)*idx elementwise
    y      = sigmoid(WoE + h_prev @ Uo + c @ Co)  # h_prev = x[:, t-1] (0 at t=0)

  So with G[b,t,v] = (x[b] @ Uo)[t,v], bias[b,v] = w0 + (c@Co)[b,v],
  delta = w1 - w0, and the binary state s_t = 1[G[t-1] + bias + delta*s_{t-1} >= theta]
  (theta = fp32 sigmoid saturation threshold; G[-1] := 0), the outputs are
      y_t = sigmoid(G[t-1] + bias + delta * s_{t-1}).
  s_t follows p0_t + (p1_t - p0_t)*s_{t-1} with p0_t = 1[G[t-1] >= theta-bias],
  p1_t = 1[G[t-1] >= theta-bias-delta], which maps exactly onto the DVE
  tensor_tensor_scan primitive (state = data0*state + data1): ONE instruction
  per batch-group. Wa, Ua, Va are mathematically dead (all-ones softmax).

Sharding: data-parallel over batch, 8 batches per core; x pre-transposed on
host to [BS, D, T] so every load is one contiguous [128, T] block.

Toolchain constraints that shaped the structure (nix walrus 2026-05):
  * ONE sync wait per instruction. Hence: warm-up consumers per engine for
    the constants, unique input tiles (no slot-recycling waits), a reserved
    DMA bookkeeping lane for the single output store (lane-first => its only
    wait is the sigmoid), and a patched Tile tail drain that splits its
    N-sem wait list into a chain of single-wait drains.
  * PE matmul psum writes only at partition bases {0, 32, 64}: two batches
    share a psum tile at bases 0/64 with stacked [Uo|pad|Co] weights.
"""

import numpy as np

import concourse.bass as bass
import concourse.mybir as mybir
import concourse.tile as _tile_mod
import concourse.tile_sem_assignment as _tsa
from concourse.tile import TileContext
from concourse.tile_scheduler import DMAInst
from concourse.vector_clock import ScopedClock
from concourse.bass_utils import run_bass_kernel_spmd

B, T, D, V = 64, 512, 1024, 28
N_CORES = 8
BS = B // N_CORES          # batches per core
KC = D // 128              # contraction chunks
NG = BS // 2               # psum pair-groups per core
F32 = mybir.dt.float32
# smallest fp32 x with 1/(1+exp(-x)) == 1.0 (24*ln2). Any value in [16, 19]
# yields indistinguishable outputs (see derivation above: a theta mismatch only
# flips idx where the NEXT sigmoid is saturated, shifting y by < 1e-6).
THETA = 16.635532333438687

CW = 64                    # packed weight chunk: 0:28 Uo, 32:60 Co, rest pad
WD = KC * CW               # column of [w0, delta, theta, theta-delta] scalars
NCONST = WD + 4

_NC_CACHE: dict = {}


# ---- Tile framework patches for the 1-wait-per-instruction walrus build ----

def _split_drain_and_barrier(self, tick_clock, wait_clock):
    """Tail drain: split its N-sem wait list into single-wait drains on SP."""
    nc = self.nc
    drain_inst = nc.sync.drain()
    wait_clock.add_sem_waits(
        drain_inst.ins, ScopedClock({None: tick_clock.global_clock})
    )
    si = drain_inst.ins.sync_info
    waits = list(si.on_wait) if si is not None and si.on_wait else []
    upds = list(si.on_update) if si is not None and si.on_update else []
    if len(waits) > 1:
        drain_inst.ins.sync_info = mybir.SyncInfo(on_wait=[waits[0]], on_update=[])
        for i, w in enumerate(waits[1:]):
            d2 = nc.sync.drain()
            last = i == len(waits) - 2
            d2.ins.sync_info = mybir.SyncInfo(
                on_wait=[w], on_update=upds if last else []
            )

    nc.all_engine_barrier()
    assert self.sems is not None
    popped = nc._tile_sem_poison_stack.pop()
    assert popped is self._sem_poison
    nc.clear_and_free_semaphores(list(self.sems.allocated().values()))
    nc.all_engine_barrier()


_tile_mod.TileContext._drain_and_barrier = _split_drain_and_barrier

# Reserve HWDGE bookkeeping lanes for the output stores (being lane-first,
# each store carries only its producer wait). All other HWDGE DMAs round-robin
# lanes 0-3.
_PIN_LANES: dict = {}
_orig_assign_tick = _tsa.TileClockTick._assign_tick


def _assign_tick_pin(self, inst):
    if isinstance(inst, DMAInst) and inst.engine != mybir.EngineType.Pool:
        if inst.name in _PIN_LANES:
            self.next_hw_dma_idx = _PIN_LANES[inst.name]
        elif self.next_hw_dma_idx >= 7:
            self.next_hw_dma_idx = 0
    return _orig_assign_tick(self, inst)


_tsa.TileClockTick._assign_tick = _assign_tick_pin


def _build_nc() -> bass.Bass:
    nc = bass.Bass()
    xt = nc.declare_dram_parameter("xt", [BS, D, T], F32, isOutput=False)
    consts = nc.declare_dram_parameter("consts", [128, NCONST], F32, isOutput=False)
    # output rows {0:28, 64:92} = batch {2g, 2g+1}, cols g*T+t; rest junk
    out = nc.declare_dram_parameter("out", [92, NG * T], F32, isOutput=True)

    with TileContext(nc) as tc:
        with (
            tc.tile_pool(name="consts_p", bufs=1) as cpool,
            tc.tile_pool(name="xin", bufs=1) as xpool,
            tc.tile_pool(name="mid", bufs=4) as mpool,
            tc.tile_pool(name="scan", bufs=2) as spool,
            tc.tile_pool(name="psum", bufs=NG, space="PSUM") as ppool,
        ):
            cb = cpool.tile([128, NCONST], F32)
            nc.sync.dma_start(out=cb[:], in_=consts[:])
            # DVE warm-up consumption so later DVE users carry no DMA wait
            junk = cpool.tile([1, 4], F32)
            nc.vector.tensor_copy(junk[:], cb[0:1, WD:WD + 4])

            # z for all 4 pair-groups side by side; zeroed so column g*T (the
            # t=0 slot) is 0 and junk rows stay finite
            z_all = cpool.tile([92, NG * T], F32)
            y_all = cpool.tile([92, NG * T], F32)
            nc.vector.memset(z_all[:], 0.0)

            ps_tiles = [
                ppool.tile([128, T], F32, tag="ps", name=f"ps{i}")
                for i in range(NG)
            ]
            # PE warm-up matmul consuming the consts DMA so no later matmul
            # needs more than one wait
            nc.tensor.matmul(
                ps_tiles[0][0:1, 0:1], cb[:, 0:1], cb[:, 0:1],
                start=True, stop=True,
            )

            # x loads: one [128, T] tile per (b, k), unique (no recycling
            # waits); 64 sequential 256 KiB direct2d transfers keep the DGE
            # ring dense at full HBM rate
            xk_tiles = {}
            for b in range(BS):
                for k in range(KC):
                    xk = xpool.tile(
                        [128, T], F32, tag=f"xk{b}_{k}", name=f"xk{b}_{k}"
                    )
                    nc.sync.dma_start(
                        out=xk[:], in_=xt[b, k * 128:(k + 1) * 128, :]
                    )
                    xk_tiles[b, k] = xk
                # one matmul per chunk: [Uo|pad|Co] stacked -> G rows at
                # base 64*(b%2), CC rows 32 above
                base = 64 * (b % 2)
                ps = ps_tiles[b // 2]
                for k in range(KC):
                    nc.tensor.matmul(
                        ps[base:base + CW, :],
                        cb[:, k * CW:(k + 1) * CW], xk_tiles[b, k][:],
                        start=(k == 0), stop=(k == KC - 1),
                    )

            for g in range(NG):
                ps = ps_tiles[g]
                zc = g * T     # this group's column block in z_all/y_all
                z0 = z_all[:, zc:zc + 1]  # always-zero column (memset)

                # bias[b] = w0 + sum_t CC.T: full-tile reduce, then shift the
                # CC rows (32:60, 96:124) down onto the G rows (0:28, 64:92)
                br = mpool.tile([124, 1], F32, tag="br")
                nc.vector.tensor_reduce(
                    out=br[:], in_=ps[0:124, :],
                    axis=mybir.AxisListType.X, op=mybir.AluOpType.add,
                )
                sb = mpool.tile([92, 1], F32, tag="sb")
                nc.vector.memset(sb[:], 0.0)
                nc.vector.tensor_copy(sb[0:28, :], br[32:60, :])
                nc.vector.tensor_copy(sb[64:92, :], br[96:124, :])
                nc.vector.tensor_scalar_add(sb[:], sb[:], cb[0:92, WD:WD + 1])
                # thresholds: tmb = theta - bias, tmbd = theta - bias - delta
                tmb = mpool.tile([92, 1], F32, tag="tmb")
                nc.vector.tensor_scalar(
                    out=tmb[:], in0=sb[:], scalar1=-1.0, scalar2=float(THETA),
                    op0=mybir.AluOpType.mult, op1=mybir.AluOpType.add,
                )
                tmbd = mpool.tile([92, 1], F32, tag="tmbd")
                nc.vector.tensor_scalar_sub(tmbd[:], tmb[:], cb[0:92, WD + 1:WD + 2])

                # p0/p1 indicators straight from psum (G rows; mid rows junk)
                p0 = spool.tile([92, T], F32, tag="p0")
                d01 = spool.tile([92, T], F32, tag="d01")
                bt = spool.tile([92, T], F32, tag="bt")
                nc.vector.tensor_scalar(
                    out=p0[:, 1:T], in0=ps[0:92, 0:T - 1], scalar1=tmb[:],
                    scalar2=None, op0=mybir.AluOpType.is_ge,
                )
                nc.vector.tensor_scalar(
                    out=p0[:, 0:1], in0=z0, scalar1=tmb[:],
                    scalar2=None, op0=mybir.AluOpType.is_ge,
                )
                nc.vector.tensor_scalar(
                    out=d01[:, 1:T], in0=ps[0:92, 0:T - 1], scalar1=tmbd[:],
                    scalar2=None, op0=mybir.AluOpType.is_ge,
                )
                nc.vector.tensor_copy(d01[:, 0:1], z0)  # any finite value
                nc.vector.tensor_sub(d01[:], d01[:], p0[:])
                # s_t = d01_t * s_{t-1} + p0_t   (exact on {0,1})
                nc.vector.tensor_tensor_scan(
                    out=bt[:], data0=d01[:], data1=p0[:], initial=0.0,
                    op0=mybir.AluOpType.mult, op1=mybir.AluOpType.add,
                )
                # z_t = G[t-1] + delta * s_{t-1}  (bias added by the sigmoid)
                nc.vector.scalar_tensor_tensor(
                    out=z_all[:, zc + 1:zc + T], in0=bt[:, 0:T - 1],
                    scalar=cb[0:92, WD + 1:WD + 2], in1=ps[0:92, 0:T - 1],
                    op0=mybir.AluOpType.mult, op1=mybir.AluOpType.add,
                )
                # y = sigmoid(z + bias)
                nc.scalar.activation(
                    out=y_all[:, zc:zc + T], in_=z_all[:, zc:zc + T],
                    func=mybir.ActivationFunctionType.Sigmoid,
                    bias=sb[:], scale=1.0,
                )
            st = nc.sync.dma_start(out=out[:], in_=y_all[:])
            _PIN_LANES[st.ins.name] = 7

    return nc


def _host_smalls(Wo, Uo, Co, emb_table):
    w0 = np.float32(emb_table[0].astype(np.float32) @ Wo[:, 0].astype(np.float32))
    w1 = np.float32(emb_table[1].astype(np.float32) @ Wo[:, 0].astype(np.float32))
    delta = np.float32(w1 - w0)
    theta = np.float32(THETA)
    uoco = np.zeros((D, CW), np.float32)
    uoco[:, 0:V] = Uo
    uoco[:, 32:32 + V] = Co
    consts = np.zeros((128, NCONST), np.float32)
    consts[:, 0:WD] = (
        uoco.reshape(KC, 128, CW).transpose(1, 0, 2).reshape(128, WD)
    )
    consts[:, WD:] = np.array(
        [w0, delta, theta, np.float32(theta - delta)], np.float32
    )
    return np.ascontiguousarray(consts)


def _in_maps(x, Wo, Uo, Co, emb_table):
    x = np.asarray(x, dtype=np.float32)
    consts = _host_smalls(
        np.asarray(Wo, np.float32), np.asarray(Uo, np.float32),
        np.asarray(Co, np.float32), np.asarray(emb_table, np.float32),
    )
    maps = []
    for c in range(N_CORES):
        xs = x[c * BS:(c + 1) * BS]                        # [BS, T, D]
        xtc = np.ascontiguousarray(xs.transpose(0, 2, 1))  # [BS, D, T]
        maps.append({"xt": xtc, "consts": consts})
    return maps


def _assemble(results):
    outs = []
    for c in range(len(results)):
        o = np.asarray(results[c]["out"]).reshape(92, NG, T)
        core = np.empty((BS, T, V), np.float32)
        core[0::2] = o[0:28].transpose(1, 2, 0)            # rows 0:28  = even b
        core[1::2] = o[64:92].transpose(1, 2, 0)           # rows 64:92 = odd b
        outs.append(core)
    return np.concatenate(outs, axis=0)                    # [B, T, V]


def _get_nc() -> bass.Bass:
    if "nc" not in _NC_CACHE:
        _NC_CACHE["nc"] = _build_nc()
    return _NC_CACHE["nc"]


def _run(inputs: dict, trace: bool = False):
    nc = _get_nc()
    maps = _in_maps(
        inputs["x"], inputs["Wo"], inputs["Uo"], inputs["Co"],
        inputs["emb_table"],
    )
    res = run_bass_kernel_spmd(nc, maps, list(range(N_CORES)), trace=trace)
    return res


def kernel(**inputs) -> np.ndarray:
    res = _run(inputs, trace=False)
    return _assemble(res.results)



# revision 2
# speedup vs baseline: 1.7247x; 1.7247x over previous
"""Trainium2 Bass kernel for nn_CascadedAttention (B=64, T=512, D=1024, V=28).

Math notes (why this is NOT a 512-step sequential scan on device):

  reference computes, per step t with carry y_prev (y_{-1} = 0):
    scores = softmax(tanh(...) @ Va, axis=-1)     # softmax over a SIZE-1 axis
                                                  # -> exactly 1.0 everywhere
    c      = einsum('btd,bt->bd', x, scores)      # -> x.sum(axis=1), step-invariant
    idx    = int32(y_prev)                        # y_prev in (0,1] -> idx in {0,1};
                                                  # idx==1 iff y_prev == 1.0 (fp32-saturated sigmoid)
    WoE    = emb_table[idx] @ Wo                  # -> w0 + (w1-w0)*idx elementwise
    y      = sigmoid(WoE + h_prev @ Uo + c @ Co)  # h_prev = x[:, t-1] (0 at t=0)

  So with G[b,t,v] = (x[b] @ Uo)[t,v], bias[b,v] = w0 + (c@Co)[b,v],
  delta = w1 - w0, and the binary state s_t = 1[G[t-1] + bias + delta*s_{t-1} >= theta]
  (theta = fp32 sigmoid saturation threshold; G[-1] := 0), the outputs are
      y_t = sigmoid(G[t-1] + bias + delta * s_{t-1}).
  s_t follows p0_t + (p1_t - p0_t)*s_{t-1} with p0_t = 1[G[t-1] >= theta-bias],
  p1_t = 1[G[t-1] >= theta-bias-delta], which maps exactly onto the DVE
  tensor_tensor_scan primitive (state = data0*state + data1): ONE instruction
  per batch-group. Wa, Ua, Va are mathematically dead (all-ones softmax).

Performance structure (fp16 revision):
  * x is shipped fp16 (halves HBM traffic — the kernel is DMA-bound) and the
    matmuls run fp16 (1 PE cycle/row vs 4 for fp32). The bias is kept exact by
    a per-(b,v) host-side correction column that absorbs both the fp16
    quantization of x inside the T-reduction and the fp16 rounding of Co:
        w0c[b,v] = w0 + c_fp32@Co - c_fp16@Co_fp16
    so the only device-side error is the per-(t,v) matmul noise of x@Uo
    (~3e-3 max), far inside the 2e-2 gate.
  * One [128, KC*T] fp16 DMA per batch (8 KiB/partition-line) instead of 64
    small transfers: the Sync-engine direct2d dispatch costs ~650 ns each.
  * Output stored fp16, one store per pair-group right after its sigmoid
    (pinned to HWDGE lanes 4..7 so each store is lane-first: single wait).

Sharding: data-parallel over batch, 8 batches per core; x pre-transposed on
host to [128, BS*KC*T] so every load is one contiguous fat-line block.

Toolchain constraints that shaped the structure (nix walrus 2026-05):
  * ONE sync wait per instruction. Hence: warm-up consumers per engine for
    the constants, unique input tiles (no slot-recycling waits), reserved
    DMA bookkeeping lanes for the output stores (lane-first => their only
    wait is the sigmoid), and a patched Tile tail drain that splits its
    N-sem wait list into a chain of single-wait drains.
  * PE matmul psum writes only at partition bases {0, 32, 64}: two batches
    share a psum tile at bases 0/64 with stacked [Uo|pad|Co] weights.
"""

import numpy as np

import concourse.bass as bass
import concourse.mybir as mybir
import concourse.tile as _tile_mod
import concourse.tile_sem_assignment as _tsa
from concourse.tile import TileContext
from concourse.tile_scheduler import DMAInst
from concourse.vector_clock import ScopedClock
from concourse.bass_utils import run_bass_kernel_spmd

B, T, D, V = 64, 512, 1024, 28
N_CORES = 8
BS = B // N_CORES          # batches per core
KC = D // 128              # contraction chunks
NG = BS // 2               # psum pair-groups per core
F32 = mybir.dt.float32
F16 = mybir.dt.float16
# smallest fp32 x with 1/(1+exp(-x)) == 1.0 (24*ln2). Any value in [16, 19]
# yields indistinguishable outputs (see derivation above: a theta mismatch only
# flips idx where the NEXT sigmoid is saturated, shifting y by < 1e-6).
THETA = 16.635532333438687

CW = 64                    # packed weight chunk: 0:28 Uo, 32:60 Co, rest pad
WD = KC * CW
XW = KC * T                # per-batch x columns in the [128, BS*XW] layout
NS = NG + 1                # scal columns: w0c per group + delta

_NC_CACHE: dict = {}


# ---- Tile framework patches for the 1-wait-per-instruction walrus build ----

def _split_drain_and_barrier(self, tick_clock, wait_clock):
    """Tail drain: split its N-sem wait list into single-wait drains on SP."""
    nc = self.nc
    drain_inst = nc.sync.drain()
    wait_clock.add_sem_waits(
        drain_inst.ins, ScopedClock({None: tick_clock.global_clock})
    )
    si = drain_inst.ins.sync_info
    waits = list(si.on_wait) if si is not None and si.on_wait else []
    upds = list(si.on_update) if si is not None and si.on_update else []
    if len(waits) > 1:
        drain_inst.ins.sync_info = mybir.SyncInfo(on_wait=[waits[0]], on_update=[])
        for i, w in enumerate(waits[1:]):
            d2 = nc.sync.drain()
            last = i == len(waits) - 2
            d2.ins.sync_info = mybir.SyncInfo(
                on_wait=[w], on_update=upds if last else []
            )

    nc.all_engine_barrier()
    assert self.sems is not None
    popped = nc._tile_sem_poison_stack.pop()
    assert popped is self._sem_poison
    nc.clear_and_free_semaphores(list(self.sems.allocated().values()))
    nc.all_engine_barrier()


_tile_mod.TileContext._drain_and_barrier = _split_drain_and_barrier

# Reserve HWDGE bookkeeping lanes 4..7 for the per-group output stores (being
# lane-first, each store carries only its producer wait). All other HWDGE DMAs
# round-robin lanes 0-3.
_PIN_LANES: dict = {}
_orig_assign_tick = _tsa.TileClockTick._assign_tick


def _assign_tick_pin(self, inst):
    if isinstance(inst, DMAInst) and inst.engine != mybir.EngineType.Pool:
        if inst.name in _PIN_LANES:
            self.next_hw_dma_idx = _PIN_LANES[inst.name]
        elif self.next_hw_dma_idx >= 4:
            self.next_hw_dma_idx = 0
    return _orig_assign_tick(self, inst)


_tsa.TileClockTick._assign_tick = _assign_tick_pin


def _build_nc() -> bass.Bass:
    nc = bass.Bass()
    xt = nc.declare_dram_parameter("xt", [128, BS * XW], F16, isOutput=False)
    wuc = nc.declare_dram_parameter("wuc", [128, WD], F16, isOutput=False)
    scal = nc.declare_dram_parameter("scal", [128, NS], F32, isOutput=False)
    # output rows {0:28, 64:92} = batch {2g, 2g+1}, cols g*T+t; rest junk
    out = nc.declare_dram_parameter("out", [92, NG * T], F16, isOutput=True)

    with TileContext(nc) as tc:
        with (
            tc.tile_pool(name="consts_p", bufs=1) as cpool,
            tc.tile_pool(name="xin", bufs=1) as xpool,
            tc.tile_pool(name="mid", bufs=4) as mpool,
            tc.tile_pool(name="scan", bufs=2) as spool,
            tc.tile_pool(name="psum", bufs=NG, space="PSUM") as ppool,
        ):
            cb = cpool.tile([128, WD], F16)
            nc.sync.dma_start(out=cb[:], in_=wuc[:])
            sc = cpool.tile([128, NS], F32)
            nc.sync.dma_start(out=sc[:], in_=scal[:])
            # DVE warm-up consumption so later DVE users carry no DMA wait
            junk = cpool.tile([1, NS], F32)
            nc.vector.tensor_copy(junk[:], sc[0:1, :])

            # z for all 4 pair-groups side by side; zeroed so column g*T (the
            # t=0 slot) is 0 and junk rows stay finite
            z_all = cpool.tile([92, NG * T], F32)
            y_all = cpool.tile([92, NG * T], F16)
            nc.vector.memset(z_all[:], 0.0)

            ps_tiles = [
                ppool.tile([128, T], F32, tag="ps", name=f"ps{i}")
                for i in range(NG)
            ]
            # PE warm-up matmul consuming the weights DMA so no later matmul
            # needs more than one wait
            nc.tensor.matmul(
                ps_tiles[0][0:1, 0:1], cb[:, 0:1], cb[:, 0:1],
                start=True, stop=True,
            )

            # x loads: one [128, KC*T] fp16 tile per batch (8 KiB per
            # partition line), unique (no recycling waits)
            for b in range(BS):
                xb = xpool.tile([128, XW], F16, tag=f"xb{b}", name=f"xb{b}")
                nc.sync.dma_start(out=xb[:], in_=xt[:, b * XW:(b + 1) * XW])
                # one matmul per chunk: [Uo|pad|Co] stacked -> G rows at
                # base 64*(b%2), CC rows 32 above
                base = 64 * (b % 2)
                ps = ps_tiles[b // 2]
                for k in range(KC):
                    nc.tensor.matmul(
                        ps[base:base + CW, :],
                        cb[:, k * CW:(k + 1) * CW],
                        xb[:, k * T:(k + 1) * T],
                        start=(k == 0), stop=(k == KC - 1),
                    )

            for g in range(NG):
                ps = ps_tiles[g]
                zc = g * T     # this group's column block in z_all/y_all
                z0 = z_all[:, zc:zc + 1]  # always-zero column (memset)

                # bias[b] = w0c + sum_t CC: full-tile reduce, then shift the
                # CC rows (32:60, 96:124) down onto the G rows (0:28, 64:92)
                br = mpool.tile([124, 1], F32, tag="br")
                nc.vector.tensor_reduce(
                    out=br[:], in_=ps[0:124, :],
                    axis=mybir.AxisListType.X, op=mybir.AluOpType.add,
                )
                sb = mpool.tile([92, 1], F32, tag="sb")
                nc.vector.memset(sb[:], 0.0)
                nc.vector.tensor_copy(sb[0:28, :], br[32:60, :])
                nc.vector.tensor_copy(sb[64:92, :], br[96:124, :])
                nc.vector.tensor_scalar_add(sb[:], sb[:], sc[0:92, g:g + 1])
                # thresholds: tmb = theta - bias, tmbd = theta - bias - delta
                tmb = mpool.tile([92, 1], F32, tag="tmb")
                nc.vector.tensor_scalar(
                    out=tmb[:], in0=sb[:], scalar1=-1.0, scalar2=float(THETA),
                    op0=mybir.AluOpType.mult, op1=mybir.AluOpType.add,
                )
                tmbd = mpool.tile([92, 1], F32, tag="tmbd")
                nc.vector.tensor_scalar_sub(tmbd[:], tmb[:], sc[0:92, NG:NG + 1])

                # p0/p1 indicators straight from psum (G rows; mid rows junk)
                p0 = spool.tile([92, T], F32, tag="p0")
                d01 = spool.tile([92, T], F32, tag="d01")
                bt = spool.tile([92, T], F32, tag="bt")
                nc.vector.tensor_scalar(
                    out=p0[:, 1:T], in0=ps[0:92, 0:T - 1], scalar1=tmb[:],
                    scalar2=None, op0=mybir.AluOpType.is_ge,
                )
                nc.vector.tensor_scalar(
                    out=p0[:, 0:1], in0=z0, scalar1=tmb[:],
                    scalar2=None, op0=mybir.AluOpType.is_ge,
                )
                nc.vector.tensor_scalar(
                    out=d01[:, 1:T], in0=ps[0:92, 0:T - 1], scalar1=tmbd[:],
                    scalar2=None, op0=mybir.AluOpType.is_ge,
                )
                nc.vector.tensor_copy(d01[:, 0:1], z0)  # any finite value
                nc.vector.tensor_sub(d01[:], d01[:], p0[:])
                # s_t = d01_t * s_{t-1} + p0_t   (exact on {0,1})
                nc.vector.tensor_tensor_scan(
                    out=bt[:], data0=d01[:], data1=p0[:], initial=0.0,
                    op0=mybir.AluOpType.mult, op1=mybir.AluOpType.add,
                )
                # z_t = G[t-1] + delta * s_{t-1}  (bias added by the sigmoid)
                nc.vector.scalar_tensor_tensor(
                    out=z_all[:, zc + 1:zc + T], in0=bt[:, 0:T - 1],
                    scalar=sc[0:92, NG:NG + 1], in1=ps[0:92, 0:T - 1],
                    op0=mybir.AluOpType.mult, op1=mybir.AluOpType.add,
                )
                # y = sigmoid(z + bias), stored fp16; store per group so only
                # the last group's chain sits in the tail
                nc.scalar.activation(
                    out=y_all[:, zc:zc + T], in_=z_all[:, zc:zc + T],
                    func=mybir.ActivationFunctionType.Sigmoid,
                    bias=sb[:], scale=1.0,
                )
                st = nc.sync.dma_start(
                    out=out[:, zc:zc + T], in_=y_all[:, zc:zc + T]
                )
                _PIN_LANES[st.ins.name] = 4 + g

    return nc


def _host_smalls(Wo, Uo, Co, emb_table):
    Wo64 = Wo[:, 0].astype(np.float64)
    w0 = float(emb_table[0].astype(np.float64) @ Wo64)
    w1 = float(emb_table[1].astype(np.float64) @ Wo64)
    delta = np.float32(np.float32(w1) - np.float32(w0))
    uoco = np.zeros((D, CW), np.float16)
    uoco[:, 0:V] = Uo.astype(np.float16)
    uoco[:, 32:32 + V] = Co.astype(np.float16)
    wuc = np.ascontiguousarray(
        uoco.reshape(KC, 128, CW).transpose(1, 0, 2).reshape(128, WD)
    )
    return wuc, w0, delta


def _in_maps(x, Wo, Uo, Co, emb_table):
    x = np.asarray(x, dtype=np.float32)
    Co = np.asarray(Co, np.float32)
    wuc, w0, delta = _host_smalls(
        np.asarray(Wo, np.float32), np.asarray(Uo, np.float32),
        Co, np.asarray(emb_table, np.float32),
    )
    Co64 = Co.astype(np.float64)
    Coh64 = Co.astype(np.float16).astype(np.float64)
    maps = []
    for c in range(N_CORES):
        xs = x[c * BS:(c + 1) * BS]                        # [BS, T, D] f32
        xh = xs.astype(np.float16)
        xtc = np.ascontiguousarray(
            xh.reshape(BS, T, KC, 128).transpose(3, 0, 2, 1)
        ).reshape(128, BS * XW)
        # exact per-(b,v) bias so the device T-reduction over fp16 x carries
        # no quantization error:  w0 + c@Co - c_fp16@Co_fp16
        c_ex = xs.sum(axis=1, dtype=np.float64)            # [BS, D]
        c_hh = xh.astype(np.float64).sum(axis=1)           # [BS, D]
        corr = w0 + c_ex @ Co64 - c_hh @ Coh64             # [BS, V]
        sc = np.zeros((128, NS), np.float32)
        for g in range(NG):
            sc[0:V, g] = corr[2 * g]
            sc[64:64 + V, g] = corr[2 * g + 1]
        sc[:, NG] = delta
        maps.append({"xt": xtc, "wuc": wuc, "scal": sc})
    return maps


def _assemble(results):
    outs = []
    for c in range(len(results)):
        o = np.asarray(results[c]["out"]).astype(np.float32)
        o = o.reshape(92, NG, T)
        core = np.empty((BS, T, V), np.float32)
        core[0::2] = o[0:28].transpose(1, 2, 0)            # rows 0:28  = even b
        core[1::2] = o[64:92].transpose(1, 2, 0)           # rows 64:92 = odd b
        outs.append(core)
    return np.concatenate(outs, axis=0)                    # [B, T, V]


def _get_nc() -> bass.Bass:
    if "nc" not in _NC_CACHE:
        _NC_CACHE["nc"] = _build_nc()
    return _NC_CACHE["nc"]


def _run(inputs: dict, trace: bool = False):
    nc = _get_nc()
    maps = _in_maps(
        inputs["x"], inputs["Wo"], inputs["Uo"], inputs["Co"],
        inputs["emb_table"],
    )
    res = run_bass_kernel_spmd(nc, maps, list(range(N_CORES)), trace=trace)
    return res


def kernel(**inputs) -> np.ndarray:
    res = _run(inputs, trace=False)
    return _assemble(res.results)
